# revision 7
# baseline (speedup 1.0000x reference)
"""Trainium2 Bass kernel for the text-CNN problem (dense_cnn).

Model: h = emb[x].reshape(B,1,L); three 1-channel 1D convs (K=3,4,5, 100
filters each) + bias + ReLU + global max-pool; concat; FC -> [B, 10].

Key identity: max_i relu(conv_i + b) == relu(b + max_i conv_i), so the
device only needs the raw per-filter max of each conv over all positions.

Device mapping (per core, 8-way shard over the 900k position axis):
  - conv as matmul: stationary [36, 128] packs 4 filters x 32 positions
    (Toeplitz bands, m = f_local*32 + r, entry [r+k, m] = w[f, 0, k]);
    moving operand is a stride-32 im2col of the signal: rhs[t, n] =
    sig[32*n + t], t in [0,36). One matmul column -> 128 useful outputs.
  - per (group, batch) "pack": 4 PSUM tiles (A1, A2, S1, S2; 2-bank
    slots, 4-deep rotation over all 8 banks).
  - drain economy (PSUM read rates: Act 1.2 G/s/lane, DVE 0.96): ScalarE
    copies A1+A2 (AW cols) to one SBUF bf16 tile cb; DVE scans S1/S2 (SW
    cols) with tensor_tensor_scan(max, max), each folding an equal span
    of cb for free (1 PSUM + 1 SBUF elem per cycle), broadcast-writing
    the running max onto one acc cell. The Act surplus (AW - SW cols of
    cb) is DMA'd to DRAM and max-reduced on the host, so every PSUM
    element is read exactly once and no engine pays a fold tax.
Host: embedding gather, im2col prep (bf16), stationaries, surplus max,
final max over r/cores, ragged-tail positions, ReLU+bias, FC.
"""

import os
import numpy as np

import concourse.bass as bass
import concourse.bacc as bacc
import concourse.mybir as mybir
from concourse.tile import TileContext
from concourse import bass_utils

import ml_dtypes

BF16 = ml_dtypes.bfloat16

# ---- problem constants (hardcoded; kernel.py must be self-contained) ----
VOCAB = 35097
WORD_DIM = 300
MAX_SENT = 3000
L = WORD_DIM * MAX_SENT          # 900000
B = 2
N_FILT = 100
KS = (3, 4, 5)
N_CLASSES = 10

N_CORES = 8
S = 32                            # positions per matmul column
TROWS = 36                        # S + max(K) - 1
GF = 4                            # filters per group
N_GROUPS = 3 * N_FILT // GF       # 75
# Per (g,b): 3516 columns split Act-drained (A1+A2 = AW) vs DVE-drained
# (S1+S2 = SW) per the 1.2/0.96 G/s engine rates; the Act surplus
# (AW - SW) is DMA'd to DRAM for a host-side max.
TWS = (945, 944, 814, 813)        # PSUM tile widths (A1, A2, S1, S2)
AW = TWS[0] + TWS[1]              # 1889 Act-copied columns
SW = TWS[2] + TWS[3]              # 1627 DVE-scanned columns
SUR = AW - SW                     # 262 surplus columns shipped to host
NCOL_B = sum(TWS)                 # 3516 columns per batch (= ceil(112500/32))
NCOL = 2 * NCOL_B                 # 7032 columns per core
P5 = L - 5 + 1                    # 899996 valid positions for K=5
CHUNK = 112500                    # positions per core (8*112500 >= P5)
CSTART_MAX = P5 - S               # 899964 max column start

ACC_COLS = N_GROUPS * 4           # 300: two accum cols per (group, batch)
SUR_COLS = N_GROUPS * 2 * SUR     # 39300 surplus columns per core


def _build_bass(n_groups=N_GROUPS, in_dt=mybir.dt.bfloat16):
    """Build the SPMD Bass module (same program on all cores).

    Per (group, batch): 4 PSUM tiles widths TWS (T0..T3; 2-bank slots, 8
    banks total, 4-slot rotation). ScalarE copies T0->cb0, T2->cb2 (bf16);
    DVE runs two independent tensor_tensor_scan(max, max) ops -- each
    consumes one PSUM and one SBUF element per cycle; each scan broadcast-
    writes its state onto one acc cell (last write = that pair's max).
    """
    nc = bacc.Bacc("TRN2", target_bir_lowering=False, debug=False,
                   num_devices=N_CORES)
    ncol = NCOL
    rhs_d = nc.dram_tensor("rhs", [TROWS, ncol], in_dt, kind="ExternalInput")
    wts_d = nc.dram_tensor("wts", [TROWS, n_groups * 128], in_dt,
                           kind="ExternalInput")
    acc_d = nc.dram_tensor("acc", [128, n_groups * 4], mybir.dt.float32,
                           kind="ExternalOutput")
    sur_d = nc.dram_tensor("sur", [128, n_groups * 2 * SUR],
                           mybir.dt.bfloat16, kind="ExternalOutput")

    bf16 = mybir.dt.bfloat16
    MAX = mybir.AluOpType.max

    with TileContext(nc) as tc:
        with tc.tile_pool(name="io", bufs=1) as io_pool, \
             tc.tile_pool(name="cb", bufs=3) as c_pool, \
             tc.tile_pool(name="ps", bufs=4, space="PSUM") as psum_pool:
            rhs = io_pool.tile([TROWS, ncol], in_dt)
            wts = io_pool.tile([TROWS, n_groups * 128], in_dt)
            acc = io_pool.tile([128, n_groups * 4], mybir.dt.float32)
            nc.sync.dma_start(rhs[:, :], rhs_d[:, :])
            nc.sync.dma_start(wts[:, :], wts_d[:, :])
            tc.strict_bb_all_engine_barrier()

            gbs = [(g, b) for g in range(n_groups) for b in range(2)]

            def mm_tile(ps, g, b, coff, tw):
                lhsT = wts[:, g * 128:(g + 1) * 128]
                col0 = b * NCOL_B + coff
                for jo, jn in ((0, 512), (512, tw - 512)):
                    nc.tensor.matmul(ps[:, jo:jo + jn], lhsT,
                                     rhs[:, col0 + jo:col0 + jo + jn],
                                     start=True, stop=True)

            def emit_a(g, b):
                a1 = psum_pool.tile([128, TWS[0]], mybir.dt.float32, tag="ps")
                mm_tile(a1, g, b, 0, TWS[0])
                a2 = psum_pool.tile([128, TWS[1]], mybir.dt.float32, tag="ps")
                mm_tile(a2, g, b, TWS[0], TWS[1])
                return (a1, a2, g, b)

            # Software-pipelined: iter k drains gb k-1 (copies first, so
            # the hoisted A-matmuls of gb k that reuse those PSUM slots
            # are never what Act waits on), then emits gb k's A-matmuls,
            # then gb k-1's S-matmuls, scans and surplus DMA.
            init = -3.0e38
            pend = emit_a(*gbs[0])
            for k in range(1, len(gbs) + 1):
                a1, a2, g, b = pend
                c0 = g * 2 + b
                cb = c_pool.tile([128, AW], bf16, tag="cbuf")
                nc.scalar.copy(cb[:, :TWS[0]], a1[:, :])
                nc.scalar.copy(cb[:, TWS[0]:], a2[:, :])
                if k < len(gbs):
                    pend = emit_a(*gbs[k])
                s1 = psum_pool.tile([128, TWS[2]], mybir.dt.float32, tag="ps")
                mm_tile(s1, g, b, AW, TWS[2])
                s2 = psum_pool.tile([128, TWS[3]], mybir.dt.float32, tag="ps")
                mm_tile(s2, g, b, AW + TWS[2], TWS[3])
                dst0 = acc[:, 2 * c0:2 * c0 + 1]
                nc.vector.tensor_tensor_scan(
                    dst0.broadcast_to([128, TWS[2]]),
                    s1[:, :], cb[:, :TWS[2]], init, op0=MAX, op1=MAX)
                dst1 = acc[:, 2 * c0 + 1:2 * c0 + 2]
                nc.vector.tensor_tensor_scan(
                    dst1.broadcast_to([128, TWS[3]]),
                    s2[:, :], cb[:, TWS[2]:SW], init, op0=MAX, op1=MAX)
                # Act surplus: ship to DRAM, host takes the max
                nc.sync.dma_start(sur_d[:, c0 * SUR:(c0 + 1) * SUR],
                                  cb[:, SW:AW])

            nc.sync.dma_start(acc_d[:, :], acc[:, :])
    nc.compile()
    return nc


# ---------------- host-side preparation ----------------

def _build_stationary(w1, w2, w3):
    """[TROWS, N_GROUPS*128]: group g covers filters 4g..4g+3 of its conv,
    column m = f_local*32 + r, entry [r+k, m] = w[f, 0, k]."""
    ws = np.zeros((TROWS, N_GROUPS * 128), np.float32)
    convs = [(np.asarray(w1, np.float32), 3),
             (np.asarray(w2, np.float32), 4),
             (np.asarray(w3, np.float32), 5)]
    g = 0
    for w, K in convs:
        for g_local in range(N_FILT // GF):
            for fl in range(GF):
                f = g_local * GF + fl
                for r in range(S):
                    ws[r:r + K, g * 128 + fl * S + r] = w[f, 0, :]
            g += 1
    return ws


def _column_starts(core):
    base = core * CHUNK
    starts = base + S * np.arange(NCOL_B)
    return np.minimum(starts, CSTART_MAX)


def _make_rhs(sig, core, dtype):
    """sig: [B, L] fp32 -> [TROWS, 2*NCOL_B] im2col for this core."""
    starts = _column_starts(core)
    cols = []
    for b in range(B):
        win = np.lib.stride_tricks.sliding_window_view(sig[b], TROWS)
        cols.append(win[starts].T)          # [TROWS, NCOL_B]
    return np.ascontiguousarray(np.concatenate(cols, axis=1)).astype(dtype)


_CACHE = {}


def _get_nc():
    if "nc" not in _CACHE:
        _CACHE["nc"] = _build_bass()
    return _CACHE["nc"]


def _device_acc(rhs_list, wts):
    """Run the bass kernel on 8 cores. rhs_list[i]: [TROWS, 2*NCOL_B].
    Returns list of (acc [128, ACC_COLS] fp32, sur [128, SUR_COLS] bf16)."""
    if os.environ.get("KERNEL_EMULATE"):
        out = []
        for rhs in rhs_list:
            acc = np.empty((128, ACC_COLS), np.float32)
            sur = np.empty((128, SUR_COLS), BF16)
            for g in range(N_GROUPS):
                pg = np.einsum("tm,tn->mn",
                               wts[:, g * 128:(g + 1) * 128].astype(np.float32),
                               rhs.astype(np.float32))  # [128, 2*NCOL_B]
                for b in range(2):
                    c0 = g * 2 + b
                    seg = pg[:, b * NCOL_B:(b + 1) * NCOL_B]
                    cb = seg[:, :AW].astype(BF16).astype(np.float32)
                    acc[:, 2 * c0] = np.maximum(
                        seg[:, AW:AW + TWS[2]].max(axis=1),
                        cb[:, :TWS[2]].max(axis=1))
                    acc[:, 2 * c0 + 1] = np.maximum(
                        seg[:, AW + TWS[2]:].max(axis=1),
                        cb[:, TWS[2]:SW].max(axis=1))
                    sur[:, c0 * SUR:(c0 + 1) * SUR] = cb[:, SW:AW]
            out.append((acc, sur))
        return out

    nc = _get_nc()
    in_maps = [{"rhs": rhs, "wts": wts} for rhs in rhs_list]
    res = bass_utils.run_bass_kernel_spmd(nc, in_maps,
                                          core_ids=list(range(N_CORES)))
    return [(r["acc"], r["sur"]) for r in res.results]


def kernel(x, emb, w1, b1, w2, b2, w3, b3, fc_w, fc_b):
    x = np.asarray(x)
    emb = np.asarray(emb, np.float32)
    sig = emb[x.reshape(-1)].reshape(B, L)          # [2, 900000] fp32

    wts = _build_stationary(w1, w2, w3).astype(BF16)
    rhs_list = [_make_rhs(sig, c, BF16) for c in range(N_CORES)]

    accs = _device_acc(rhs_list, wts)

    # acc cells + DMA'd surplus columns -> per-batch per-filter maxes
    conv_max = np.full((B, 3 * N_FILT), -np.inf, np.float32)
    for acc, sur in accs:
        a = acc.reshape(128, N_GROUPS, 2, 2)
        sm = sur.astype(np.float32).reshape(128, N_GROUPS, 2, SUR).max(axis=3)
        for b in range(B):
            mb = np.maximum(a[:, :, b, :].max(axis=2), sm[:, :, b])  # [128, 75]
            # rows m = f_local*32 + r -> [GF, S, N_GROUPS] -> max over r
            mb = mb.reshape(GF, S, N_GROUPS).max(axis=1)           # [GF, 75]
            # filter id = group_base + (g_local*GF + f_local)
            mb = mb.T.reshape(3, N_FILT // GF, GF).reshape(3 * N_FILT)
            conv_max[b] = np.maximum(conv_max[b], mb)

    # ragged tail positions not covered on device (fp32 host math)
    w1a = np.asarray(w1, np.float32)
    w2a = np.asarray(w2, np.float32)
    for b in range(B):
        for p in (L - 3 + 1 - 1, L - 3 + 1 - 2):   # 899997, 899996 (K=3)
            if p > P5 - 1:
                v = sig[b, p:p + 3] @ w1a[:, 0, :].T
                conv_max[b, :N_FILT] = np.maximum(conv_max[b, :N_FILT], v)
        p = L - 4 + 1 - 1                           # 899996 (K=4)
        if p > P5 - 1:
            v = sig[b, p:p + 4] @ w2a[:, 0, :].T
            conv_max[b, N_FILT:2 * N_FILT] = \
                np.maximum(conv_max[b, N_FILT:2 * N_FILT], v)

    bias = np.concatenate([np.asarray(b1, np.float32),
                           np.asarray(b2, np.float32),
                           np.asarray(b3, np.float32)])
    feats = np.maximum(conv_max + bias[None, :], 0.0)
    out = feats @ np.asarray(fc_w, np.float32).T + np.asarray(fc_b, np.float32)
    return out.astype(np.float32)



# revision 8
# speedup vs baseline: 1.0042x; 1.0042x over previous
"""Trainium2 Bass kernel for the text-CNN problem (dense_cnn).

Model: h = emb[x].reshape(B,1,L); three 1-channel 1D convs (K=3,4,5, 100
filters each) + bias + ReLU + global max-pool; concat; FC -> [B, 10].

Key identity: max_i relu(conv_i + b) == relu(b + max_i conv_i), so the
device only needs the raw per-filter max of each conv over all positions.

Device mapping (per core, 8-way shard over the 900k position axis):
  - conv as matmul: stationary [36, 128] packs 4 filters x 32 positions
    (Toeplitz bands, m = f_local*32 + r, entry [r+k, m] = w[f, 0, k]);
    moving operand is a stride-32 im2col of the signal: rhs[t, n] =
    sig[32*n + t], t in [0,36). One matmul column -> 128 useful outputs.
  - per (group, batch) "pack": 4 PSUM tiles (A1, A2, S1, S2; 2-bank
    slots, 4-deep rotation over all 8 banks).
  - drain economy (PSUM read rates: Act 1.2 G/s/lane, DVE 0.96): ScalarE
    copies A1+A2 (AW cols) to one SBUF bf16 tile cb; DVE scans S1/S2 (SW
    cols) with tensor_tensor_scan(max, max), each folding an equal span
    of cb for free (1 PSUM + 1 SBUF elem per cycle), broadcast-writing
    the running max onto one acc cell. The Act surplus (AW - SW cols of
    cb) is DMA'd to DRAM and max-reduced on the host, so every PSUM
    element is read exactly once and no engine pays a fold tax.
Host: embedding gather, im2col prep (bf16), stationaries, surplus max,
final max over r/cores, ragged-tail positions, ReLU+bias, FC.
"""

import os
import numpy as np

import concourse.bass as bass
import concourse.bacc as bacc
import concourse.mybir as mybir
from concourse.tile import TileContext
from concourse import bass_utils

import ml_dtypes

BF16 = ml_dtypes.bfloat16

# ---- problem constants (hardcoded; kernel.py must be self-contained) ----
VOCAB = 35097
WORD_DIM = 300
MAX_SENT = 3000
L = WORD_DIM * MAX_SENT          # 900000
B = 2
N_FILT = 100
KS = (3, 4, 5)
N_CLASSES = 10

N_CORES = 8
S = 32                            # positions per matmul column
TROWS = 36                        # S + max(K) - 1
GF = 4                            # filters per group
N_GROUPS = 3 * N_FILT // GF       # 75
# Per (g,b): 3516 columns split Act-drained (A1+A2 = AW) vs DVE-drained
# (S1+S2 = SW) per the 1.2/0.96 G/s engine rates; the Act surplus
# (AW - SW) is DMA'd to DRAM for a host-side max.
TWS = (945, 944, 814, 813)        # PSUM tile widths (A1, A2, S1, S2)
AW = TWS[0] + TWS[1]              # 1889 Act-copied columns
SW = TWS[2] + TWS[3]              # 1627 DVE-scanned columns
SUR = AW - SW                     # 262 surplus columns shipped to host
NCOL_B = sum(TWS)                 # 3516 columns per batch (= ceil(112500/32))
NCOL = 2 * NCOL_B                 # 7032 columns per core
P5 = L - 5 + 1                    # 899996 valid positions for K=5
CHUNK = 112500                    # positions per core (8*112500 >= P5)
CSTART_MAX = P5 - S               # 899964 max column start

ACC_COLS = N_GROUPS * 4           # 300: two accum cols per (group, batch)
SUR_COLS = N_GROUPS * 2 * SUR     # 39300 surplus columns per core


def _build_bass(n_groups=N_GROUPS, in_dt=mybir.dt.bfloat16):
    """Build the SPMD Bass module (same program on all cores).

    Per (group, batch): 4 PSUM tiles widths TWS (T0..T3; 2-bank slots, 8
    banks total, 4-slot rotation). ScalarE copies T0->cb0, T2->cb2 (bf16);
    DVE runs two independent tensor_tensor_scan(max, max) ops -- each
    consumes one PSUM and one SBUF element per cycle; each scan broadcast-
    writes its state onto one acc cell (last write = that pair's max).
    """
    nc = bacc.Bacc("TRN2", target_bir_lowering=False, debug=False,
                   num_devices=N_CORES)
    ncol = NCOL
    rhs_d = nc.dram_tensor("rhs", [TROWS, ncol], in_dt, kind="ExternalInput")
    wts_d = nc.dram_tensor("wts", [TROWS, n_groups * 128], in_dt,
                           kind="ExternalInput")
    acc_d = nc.dram_tensor("acc", [128, n_groups * 4], mybir.dt.float32,
                           kind="ExternalOutput")
    sur_d = nc.dram_tensor("sur", [128, n_groups * 2 * SUR],
                           mybir.dt.bfloat16, kind="ExternalOutput")

    bf16 = mybir.dt.bfloat16
    MAX = mybir.AluOpType.max

    with TileContext(nc) as tc:
        with tc.tile_pool(name="io", bufs=1) as io_pool, \
             tc.tile_pool(name="cb", bufs=3) as c_pool, \
             tc.tile_pool(name="ps", bufs=4, space="PSUM") as psum_pool:
            rhs = io_pool.tile([TROWS, ncol], in_dt)
            wts = io_pool.tile([TROWS, n_groups * 128], in_dt)
            acc = io_pool.tile([128, n_groups * 4], mybir.dt.float32)
            nc.sync.dma_start(rhs[:, :], rhs_d[:, :])
            nc.sync.dma_start(wts[:, :], wts_d[:, :])
            tc.strict_bb_all_engine_barrier()

            gbs = [(g, b) for g in range(n_groups) for b in range(2)]

            def mm_tile(ps, g, b, coff, tw):
                lhsT = wts[:, g * 128:(g + 1) * 128]
                col0 = b * NCOL_B + coff
                for jo, jn in ((0, 512), (512, tw - 512)):
                    nc.tensor.matmul(ps[:, jo:jo + jn], lhsT,
                                     rhs[:, col0 + jo:col0 + jo + jn],
                                     start=True, stop=True)

            # Software-pipelined: iter k drains gb k-1 (copies first), then
            # interleaves gb k's A-matmuls with gb k-1's S-matmuls so PE
            # visits tiles in dependency-readiness order: A1_k's slot frees
            # when copy1_{k-1} ends (early), A2_k's when copy2_{k-1} ends
            # (period-end) — A2 goes last so it never blocks the S-matmuls
            # the scans need.
            init = -3.0e38

            def emit_a1(g, b):
                a1 = psum_pool.tile([128, TWS[0]], mybir.dt.float32, tag="ps")
                mm_tile(a1, g, b, 0, TWS[0])
                return a1

            def emit_a2(g, b):
                a2 = psum_pool.tile([128, TWS[1]], mybir.dt.float32, tag="ps")
                mm_tile(a2, g, b, TWS[0], TWS[1])
                return a2

            pend = (emit_a1(*gbs[0]), emit_a2(*gbs[0])) + gbs[0]
            for k in range(1, len(gbs) + 1):
                a1, a2, g, b = pend
                c0 = g * 2 + b
                cb = c_pool.tile([128, AW], bf16, tag="cbuf")
                nc.scalar.copy(cb[:, :TWS[0]], a1[:, :])
                nc.scalar.copy(cb[:, TWS[0]:], a2[:, :])
                if k < len(gbs):
                    na1 = emit_a1(*gbs[k])
                s1 = psum_pool.tile([128, TWS[2]], mybir.dt.float32, tag="ps")
                mm_tile(s1, g, b, AW, TWS[2])
                s2 = psum_pool.tile([128, TWS[3]], mybir.dt.float32, tag="ps")
                mm_tile(s2, g, b, AW + TWS[2], TWS[3])
                if k < len(gbs):
                    pend = (na1, emit_a2(*gbs[k])) + gbs[k]
                dst0 = acc[:, 2 * c0:2 * c0 + 1]
                nc.vector.tensor_tensor_scan(
                    dst0.broadcast_to([128, TWS[2]]),
                    s1[:, :], cb[:, :TWS[2]], init, op0=MAX, op1=MAX)
                dst1 = acc[:, 2 * c0 + 1:2 * c0 + 2]
                nc.vector.tensor_tensor_scan(
                    dst1.broadcast_to([128, TWS[3]]),
                    s2[:, :], cb[:, TWS[2]:SW], init, op0=MAX, op1=MAX)
                # Act surplus: ship to DRAM, host takes the max
                nc.sync.dma_start(sur_d[:, c0 * SUR:(c0 + 1) * SUR],
                                  cb[:, SW:AW])

            nc.sync.dma_start(acc_d[:, :], acc[:, :])
    nc.compile()
    return nc


# ---------------- host-side preparation ----------------

def _build_stationary(w1, w2, w3):
    """[TROWS, N_GROUPS*128]: group g covers filters 4g..4g+3 of its conv,
    column m = f_local*32 + r, entry [r+k, m] = w[f, 0, k]."""
    ws = np.zeros((TROWS, N_GROUPS * 128), np.float32)
    convs = [(np.asarray(w1, np.float32), 3),
             (np.asarray(w2, np.float32), 4),
             (np.asarray(w3, np.float32), 5)]
    g = 0
    for w, K in convs:
        for g_local in range(N_FILT // GF):
            for fl in range(GF):
                f = g_local * GF + fl
                for r in range(S):
                    ws[r:r + K, g * 128 + fl * S + r] = w[f, 0, :]
            g += 1
    return ws


def _column_starts(core):
    base = core * CHUNK
    starts = base + S * np.arange(NCOL_B)
    return np.minimum(starts, CSTART_MAX)


def _make_rhs(sig, core, dtype):
    """sig: [B, L] fp32 -> [TROWS, 2*NCOL_B] im2col for this core."""
    starts = _column_starts(core)
    cols = []
    for b in range(B):
        win = np.lib.stride_tricks.sliding_window_view(sig[b], TROWS)
        cols.append(win[starts].T)          # [TROWS, NCOL_B]
    return np.ascontiguousarray(np.concatenate(cols, axis=1)).astype(dtype)


_CACHE = {}


def _get_nc():
    if "nc" not in _CACHE:
        _CACHE["nc"] = _build_bass()
    return _CACHE["nc"]


def _device_acc(rhs_list, wts):
    """Run the bass kernel on 8 cores. rhs_list[i]: [TROWS, 2*NCOL_B].
    Returns list of (acc [128, ACC_COLS] fp32, sur [128, SUR_COLS] bf16)."""
    if os.environ.get("KERNEL_EMULATE"):
        out = []
        for rhs in rhs_list:
            acc = np.empty((128, ACC_COLS), np.float32)
            sur = np.empty((128, SUR_COLS), BF16)
            for g in range(N_GROUPS):
                pg = np.einsum("tm,tn->mn",
                               wts[:, g * 128:(g + 1) * 128].astype(np.float32),
                               rhs.astype(np.float32))  # [128, 2*NCOL_B]
                for b in range(2):
                    c0 = g * 2 + b
                    seg = pg[:, b * NCOL_B:(b + 1) * NCOL_B]
                    cb = seg[:, :AW].astype(BF16).astype(np.float32)
                    acc[:, 2 * c0] = np.maximum(
                        seg[:, AW:AW + TWS[2]].max(axis=1),
                        cb[:, :TWS[2]].max(axis=1))
                    acc[:, 2 * c0 + 1] = np.maximum(
                        seg[:, AW + TWS[2]:].max(axis=1),
                        cb[:, TWS[2]:SW].max(axis=1))
                    sur[:, c0 * SUR:(c0 + 1) * SUR] = cb[:, SW:AW]
            out.append((acc, sur))
        return out

    nc = _get_nc()
    in_maps = [{"rhs": rhs, "wts": wts} for rhs in rhs_list]
    res = bass_utils.run_bass_kernel_spmd(nc, in_maps,
                                          core_ids=list(range(N_CORES)))
    return [(r["acc"], r["sur"]) for r in res.results]


def kernel(x, emb, w1, b1, w2, b2, w3, b3, fc_w, fc_b):
    x = np.asarray(x)
    emb = np.asarray(emb, np.float32)
    sig = emb[x.reshape(-1)].reshape(B, L)          # [2, 900000] fp32

    wts = _build_stationary(w1, w2, w3).astype(BF16)
    rhs_list = [_make_rhs(sig, c, BF16) for c in range(N_CORES)]

    accs = _device_acc(rhs_list, wts)

    # acc cells + DMA'd surplus columns -> per-batch per-filter maxes
    conv_max = np.full((B, 3 * N_FILT), -np.inf, np.float32)
    for acc, sur in accs:
        a = acc.reshape(128, N_GROUPS, 2, 2)
        sm = sur.astype(np.float32).reshape(128, N_GROUPS, 2, SUR).max(axis=3)
        for b in range(B):
            mb = np.maximum(a[:, :, b, :].max(axis=2), sm[:, :, b])  # [128, 75]
            # rows m = f_local*32 + r -> [GF, S, N_GROUPS] -> max over r
            mb = mb.reshape(GF, S, N_GROUPS).max(axis=1)           # [GF, 75]
            # filter id = group_base + (g_local*GF + f_local)
            mb = mb.T.reshape(3, N_FILT // GF, GF).reshape(3 * N_FILT)
            conv_max[b] = np.maximum(conv_max[b], mb)

    # ragged tail positions not covered on device (fp32 host math)
    w1a = np.asarray(w1, np.float32)
    w2a = np.asarray(w2, np.float32)
    for b in range(B):
        for p in (L - 3 + 1 - 1, L - 3 + 1 - 2):   # 899997, 899996 (K=3)
            if p > P5 - 1:
                v = sig[b, p:p + 3] @ w1a[:, 0, :].T
                conv_max[b, :N_FILT] = np.maximum(conv_max[b, :N_FILT], v)
        p = L - 4 + 1 - 1                           # 899996 (K=4)
        if p > P5 - 1:
            v = sig[b, p:p + 4] @ w2a[:, 0, :].T
            conv_max[b, N_FILT:2 * N_FILT] = \
                np.maximum(conv_max[b, N_FILT:2 * N_FILT], v)

    bias = np.concatenate([np.asarray(b1, np.float32),
                           np.asarray(b2, np.float32),
                           np.asarray(b3, np.float32)])
    feats = np.maximum(conv_max + bias[None, :], 0.0)
    out = feats @ np.asarray(fc_w, np.float32).T + np.asarray(fc_b, np.float32)
    return out.astype(np.float32)



# revision 9
# speedup vs baseline: 1.0047x; 1.0005x over previous
"""Trainium2 Bass kernel for the text-CNN problem (dense_cnn).

Model: h = emb[x].reshape(B,1,L); three 1-channel 1D convs (K=3,4,5, 100
filters each) + bias + ReLU + global max-pool; concat; FC -> [B, 10].

Key identity: max_i relu(conv_i + b) == relu(b + max_i conv_i), so the
device only needs the raw per-filter max of each conv over all positions.

Device mapping (per core, 8-way shard over the 900k position axis):
  - conv as matmul: stationary [36, 128] packs 4 filters x 32 positions
    (Toeplitz bands, m = f_local*32 + r, entry [r+k, m] = w[f, 0, k]);
    moving operand is a stride-32 im2col of the signal: rhs[t, n] =
    sig[32*n + t], t in [0,36). One matmul column -> 128 useful outputs.
  - per (group, batch) "pack": 4 PSUM tiles (A1, A2, S1, S2; 2-bank
    slots, 4-deep rotation over all 8 banks).
  - drain economy (PSUM read rates: Act 1.2 G/s/lane, DVE 0.96): ScalarE
    copies A1+A2 (AW cols) to one SBUF bf16 tile cb; DVE scans S1/S2 (SW
    cols) with tensor_tensor_scan(max, max), each folding an equal span
    of cb for free (1 PSUM + 1 SBUF elem per cycle), broadcast-writing
    the running max onto one acc cell. The Act surplus (AW - SW cols of
    cb) is DMA'd to DRAM and max-reduced on the host, so every PSUM
    element is read exactly once and no engine pays a fold tax.
Host: embedding gather, im2col prep (bf16), stationaries, surplus max,
final max over r/cores, ragged-tail positions, ReLU+bias, FC.
"""

import os
import numpy as np

import concourse.bass as bass
import concourse.bacc as bacc
import concourse.mybir as mybir
from concourse.tile import TileContext
from concourse import bass_utils

import ml_dtypes

BF16 = ml_dtypes.bfloat16

# ---- problem constants (hardcoded; kernel.py must be self-contained) ----
VOCAB = 35097
WORD_DIM = 300
MAX_SENT = 3000
L = WORD_DIM * MAX_SENT          # 900000
B = 2
N_FILT = 100
KS = (3, 4, 5)
N_CLASSES = 10

N_CORES = 8
S = 32                            # positions per matmul column
TROWS = 36                        # S + max(K) - 1
GF = 4                            # filters per group
N_GROUPS = 3 * N_FILT // GF       # 75
# Per (g,b): 3516 columns split Act-drained (A1+A2 = AW) vs DVE-drained
# (S1+S2 = SW) per the 1.2/0.96 G/s engine rates; the Act surplus
# (AW - SW) is DMA'd to DRAM for a host-side max.
TWS = (945, 944, 814, 813)        # PSUM tile widths (A1, A2, S1, S2)
AW = TWS[0] + TWS[1]              # 1889 Act-copied columns
SW = TWS[2] + TWS[3]              # 1627 DVE-scanned columns
SUR = AW - SW                     # 262 surplus columns shipped to host
NCOL_B = sum(TWS)                 # 3516 columns per batch (= ceil(112500/32))
NCOL = 2 * NCOL_B                 # 7032 columns per core
P5 = L - 5 + 1                    # 899996 valid positions for K=5
CHUNK = 112500                    # positions per core (8*112500 >= P5)
CSTART_MAX = P5 - S               # 899964 max column start

ACC_COLS = N_GROUPS * 4           # 300: two accum cols per (group, batch)
SUR_COLS = N_GROUPS * 2 * SUR     # 39300 surplus columns per core


def _build_bass(n_groups=N_GROUPS, in_dt=mybir.dt.bfloat16):
    """Build the SPMD Bass module (same program on all cores).

    Per (group, batch): 4 PSUM tiles widths TWS (T0..T3; 2-bank slots, 8
    banks total, 4-slot rotation). ScalarE copies T0->cb0, T2->cb2 (bf16);
    DVE runs two independent tensor_tensor_scan(max, max) ops -- each
    consumes one PSUM and one SBUF element per cycle; each scan broadcast-
    writes its state onto one acc cell (last write = that pair's max).
    """
    nc = bacc.Bacc("TRN2", target_bir_lowering=False, debug=False,
                   num_devices=N_CORES)
    ncol = NCOL
    rhs_d = nc.dram_tensor("rhs", [TROWS, ncol], in_dt, kind="ExternalInput")
    wts_d = nc.dram_tensor("wts", [TROWS, n_groups * 128], in_dt,
                           kind="ExternalInput")
    acc_d = nc.dram_tensor("acc", [128, n_groups * 4], mybir.dt.float32,
                           kind="ExternalOutput")
    sur_d = nc.dram_tensor("sur", [128, n_groups * 2 * SUR],
                           mybir.dt.bfloat16, kind="ExternalOutput")

    bf16 = mybir.dt.bfloat16
    MAX = mybir.AluOpType.max

    with TileContext(nc) as tc:
        with tc.tile_pool(name="io", bufs=1) as io_pool, \
             tc.tile_pool(name="cb", bufs=4) as c_pool, \
             tc.tile_pool(name="ps", bufs=4, space="PSUM") as psum_pool:
            rhs = io_pool.tile([TROWS, ncol], in_dt)
            wts = io_pool.tile([TROWS, n_groups * 128], in_dt)
            acc = io_pool.tile([128, n_groups * 4], mybir.dt.float32)
            nc.sync.dma_start(rhs[:, :], rhs_d[:, :])
            nc.sync.dma_start(wts[:, :], wts_d[:, :])
            tc.strict_bb_all_engine_barrier()

            gbs = [(g, b) for g in range(n_groups) for b in range(2)]

            def mm_tile(ps, g, b, coff, tw):
                lhsT = wts[:, g * 128:(g + 1) * 128]
                col0 = b * NCOL_B + coff
                for jo, jn in ((0, 512), (512, tw - 512)):
                    nc.tensor.matmul(ps[:, jo:jo + jn], lhsT,
                                     rhs[:, col0 + jo:col0 + jo + jn],
                                     start=True, stop=True)

            # Software-pipelined: iter k drains gb k-1 (copies first), then
            # interleaves gb k's A-matmuls with gb k-1's S-matmuls so PE
            # visits tiles in dependency-readiness order: A1_k's slot frees
            # when copy1_{k-1} ends (early), A2_k's when copy2_{k-1} ends
            # (period-end) — A2 goes last so it never blocks the S-matmuls
            # the scans need.
            init = -3.0e38

            def emit_a1(g, b):
                a1 = psum_pool.tile([128, TWS[0]], mybir.dt.float32, tag="ps")
                mm_tile(a1, g, b, 0, TWS[0])
                return a1

            def emit_a2(g, b):
                a2 = psum_pool.tile([128, TWS[1]], mybir.dt.float32, tag="ps")
                mm_tile(a2, g, b, TWS[0], TWS[1])
                return a2

            pend = (emit_a1(*gbs[0]), emit_a2(*gbs[0])) + gbs[0]
            for k in range(1, len(gbs) + 1):
                a1, a2, g, b = pend
                c0 = g * 2 + b
                cb = c_pool.tile([128, AW], bf16, tag="cbuf")
                nc.scalar.copy(cb[:, :TWS[0]], a1[:, :])
                nc.scalar.copy(cb[:, TWS[0]:], a2[:, :])
                # Act surplus: ship to DRAM (gpsimd SWDGE queue; SP and the
                # hwdge queues would serialize behind the copy2 sem wait),
                # host takes the max
                nc.gpsimd.dma_start(sur_d[:, c0 * SUR:(c0 + 1) * SUR],
                                    cb[:, SW:AW])
                if k < len(gbs):
                    na1 = emit_a1(*gbs[k])
                s1 = psum_pool.tile([128, TWS[2]], mybir.dt.float32, tag="ps")
                mm_tile(s1, g, b, AW, TWS[2])
                s2 = psum_pool.tile([128, TWS[3]], mybir.dt.float32, tag="ps")
                mm_tile(s2, g, b, AW + TWS[2], TWS[3])
                if k < len(gbs):
                    pend = (na1, emit_a2(*gbs[k])) + gbs[k]
                dst0 = acc[:, 2 * c0:2 * c0 + 1]
                nc.vector.tensor_tensor_scan(
                    dst0.broadcast_to([128, TWS[2]]),
                    s1[:, :], cb[:, :TWS[2]], init, op0=MAX, op1=MAX)
                dst1 = acc[:, 2 * c0 + 1:2 * c0 + 2]
                nc.vector.tensor_tensor_scan(
                    dst1.broadcast_to([128, TWS[3]]),
                    s2[:, :], cb[:, TWS[2]:SW], init, op0=MAX, op1=MAX)

            nc.sync.dma_start(acc_d[:, :], acc[:, :])
    nc.compile()
    return nc


# ---------------- host-side preparation ----------------

def _build_stationary(w1, w2, w3):
    """[TROWS, N_GROUPS*128]: group g covers filters 4g..4g+3 of its conv,
    column m = f_local*32 + r, entry [r+k, m] = w[f, 0, k]."""
    ws = np.zeros((TROWS, N_GROUPS * 128), np.float32)
    convs = [(np.asarray(w1, np.float32), 3),
             (np.asarray(w2, np.float32), 4),
             (np.asarray(w3, np.float32), 5)]
    g = 0
    for w, K in convs:
        for g_local in range(N_FILT // GF):
            for fl in range(GF):
                f = g_local * GF + fl
                for r in range(S):
                    ws[r:r + K, g * 128 + fl * S + r] = w[f, 0, :]
            g += 1
    return ws


def _column_starts(core):
    base = core * CHUNK
    starts = base + S * np.arange(NCOL_B)
    return np.minimum(starts, CSTART_MAX)


def _make_rhs(sig, core, dtype):
    """sig: [B, L] fp32 -> [TROWS, 2*NCOL_B] im2col for this core."""
    starts = _column_starts(core)
    cols = []
    for b in range(B):
        win = np.lib.stride_tricks.sliding_window_view(sig[b], TROWS)
        cols.append(win[starts].T)          # [TROWS, NCOL_B]
    return np.ascontiguousarray(np.concatenate(cols, axis=1)).astype(dtype)


_CACHE = {}


def _get_nc():
    if "nc" not in _CACHE:
        _CACHE["nc"] = _build_bass()
    return _CACHE["nc"]


def _device_acc(rhs_list, wts):
    """Run the bass kernel on 8 cores. rhs_list[i]: [TROWS, 2*NCOL_B].
    Returns list of (acc [128, ACC_COLS] fp32, sur [128, SUR_COLS] bf16)."""
    if os.environ.get("KERNEL_EMULATE"):
        out = []
        for rhs in rhs_list:
            acc = np.empty((128, ACC_COLS), np.float32)
            sur = np.empty((128, SUR_COLS), BF16)
            for g in range(N_GROUPS):
                pg = np.einsum("tm,tn->mn",
                               wts[:, g * 128:(g + 1) * 128].astype(np.float32),
                               rhs.astype(np.float32))  # [128, 2*NCOL_B]
                for b in range(2):
                    c0 = g * 2 + b
                    seg = pg[:, b * NCOL_B:(b + 1) * NCOL_B]
                    cb = seg[:, :AW].astype(BF16).astype(np.float32)
                    acc[:, 2 * c0] = np.maximum(
                        seg[:, AW:AW + TWS[2]].max(axis=1),
                        cb[:, :TWS[2]].max(axis=1))
                    acc[:, 2 * c0 + 1] = np.maximum(
                        seg[:, AW + TWS[2]:].max(axis=1),
                        cb[:, TWS[2]:SW].max(axis=1))
                    sur[:, c0 * SUR:(c0 + 1) * SUR] = cb[:, SW:AW]
            out.append((acc, sur))
        return out

    nc = _get_nc()
    in_maps = [{"rhs": rhs, "wts": wts} for rhs in rhs_list]
    res = bass_utils.run_bass_kernel_spmd(nc, in_maps,
                                          core_ids=list(range(N_CORES)))
    return [(r["acc"], r["sur"]) for r in res.results]


def kernel(x, emb, w1, b1, w2, b2, w3, b3, fc_w, fc_b):
    x = np.asarray(x)
    emb = np.asarray(emb, np.float32)
    sig = emb[x.reshape(-1)].reshape(B, L)          # [2, 900000] fp32

    wts = _build_stationary(w1, w2, w3).astype(BF16)
    rhs_list = [_make_rhs(sig, c, BF16) for c in range(N_CORES)]

    accs = _device_acc(rhs_list, wts)

    # acc cells + DMA'd surplus columns -> per-batch per-filter maxes
    conv_max = np.full((B, 3 * N_FILT), -np.inf, np.float32)
    for acc, sur in accs:
        a = acc.reshape(128, N_GROUPS, 2, 2)
        sm = sur.astype(np.float32).reshape(128, N_GROUPS, 2, SUR).max(axis=3)
        for b in range(B):
            mb = np.maximum(a[:, :, b, :].max(axis=2), sm[:, :, b])  # [128, 75]
            # rows m = f_local*32 + r -> [GF, S, N_GROUPS] -> max over r
            mb = mb.reshape(GF, S, N_GROUPS).max(axis=1)           # [GF, 75]
            # filter id = group_base + (g_local*GF + f_local)
            mb = mb.T.reshape(3, N_FILT // GF, GF).reshape(3 * N_FILT)
            conv_max[b] = np.maximum(conv_max[b], mb)

    # ragged tail positions not covered on device (fp32 host math)
    w1a = np.asarray(w1, np.float32)
    w2a = np.asarray(w2, np.float32)
    for b in range(B):
        for p in (L - 3 + 1 - 1, L - 3 + 1 - 2):   # 899997, 899996 (K=3)
            if p > P5 - 1:
                v = sig[b, p:p + 3] @ w1a[:, 0, :].T
                conv_max[b, :N_FILT] = np.maximum(conv_max[b, :N_FILT], v)
        p = L - 4 + 1 - 1                           # 899996 (K=4)
        if p > P5 - 1:
            v = sig[b, p:p + 4] @ w2a[:, 0, :].T
            conv_max[b, N_FILT:2 * N_FILT] = \
                np.maximum(conv_max[b, N_FILT:2 * N_FILT], v)

    bias = np.concatenate([np.asarray(b1, np.float32),
                           np.asarray(b2, np.float32),
                           np.asarray(b3, np.float32)])
    feats = np.maximum(conv_max + bias[None, :], 0.0)
    out = feats @ np.asarray(fc_w, np.float32).T + np.asarray(fc_b, np.float32)
    return out.astype(np.float32)



# revision 12
# speedup vs baseline: 1.0756x; 1.0706x over previous
"""Trainium2 Bass kernel for the text-CNN problem (dense_cnn).

Model: h = emb[x].reshape(B,1,L); three 1-channel 1D convs (K=3,4,5, 100
filters each) + bias + ReLU + global max-pool; concat; FC -> [B, 10].

Key identity: max_i relu(conv_i + b) == relu(b + max_i conv_i), so the
device only needs the raw per-filter max of each conv over all positions.

Device mapping (per core, 8-way shard over the 900k position axis):
  - conv as matmul: stationary [36, 128] packs 4 filters x 32 positions
    (Toeplitz bands, m = f_local*32 + r, entry [r+k, m] = w[f, 0, k]);
    moving operand is a stride-32 im2col of the signal: rhs[t, n] =
    sig[32*n + t], t in [0,36). One matmul column -> 128 useful outputs.
  - per (group, batch) "pack": 4 PSUM tiles (A1, A2, S1, S2; 2-bank
    slots, 4-deep rotation over all 8 banks).
  - drain economy (PSUM read rates: Act 1.2 G/s/lane, DVE 0.96): ScalarE
    copies A1+A2 (AW cols) to one SBUF bf16 tile cb; DVE scans S1/S2 (SW
    cols) with tensor_tensor_scan(max, max), each folding an equal span
    of cb for free (1 PSUM + 1 SBUF elem per cycle), broadcast-writing
    the running max onto one acc cell. The Act surplus (AW - SW cols of
    cb) is DMA'd to DRAM and max-reduced on the host, so every PSUM
    element is read exactly once and no engine pays a fold tax.
Host: embedding gather, im2col prep (bf16), stationaries, surplus max,
final max over r/cores, ragged-tail positions, ReLU+bias, FC.
"""

import os
import numpy as np

import concourse.bass as bass
import concourse.bacc as bacc
import concourse.mybir as mybir
from concourse.tile import TileContext
from concourse import bass_utils

import ml_dtypes

BF16 = ml_dtypes.bfloat16

# ---- problem constants (hardcoded; kernel.py must be self-contained) ----
VOCAB = 35097
WORD_DIM = 300
MAX_SENT = 3000
L = WORD_DIM * MAX_SENT          # 900000
B = 2
N_FILT = 100
KS = (3, 4, 5)
N_CLASSES = 10

N_CORES = 8
S = 32                            # positions per matmul column
TROWS = 36                        # S + max(K) - 1
GF = 4                            # filters per group
N_GROUPS = 3 * N_FILT // GF       # 75
# Per (g,b): 3516 columns split Act-drained (A1+A2 = AW) vs DVE-drained
# (S1+S2 = SW) per the 1.2/0.96 G/s engine rates; the Act surplus
# (AW - SW) is DMA'd to DRAM for a host-side max.
# scan_a pairs cb[0:S1W] (inside copy1's range) and scan_b pairs
# cb[A1W:A1W+S2W] (inside copy2's range) so each scan depends on exactly
# one copy; the surplus (tail of copy2) is DMA'd to the host.
TWS = (1024, 866, 1024, 602)      # PSUM tile widths (A1, A2, S1, S2)
AW = TWS[0] + TWS[1]              # 1890 Act-copied columns
SW = TWS[2] + TWS[3]              # 1626 DVE-scanned columns
SUR = AW - SW                     # 264 surplus columns shipped to host
NCOL_B = sum(TWS)                 # 3516 columns per batch (= ceil(112500/32))
NCOL = 2 * NCOL_B                 # 7032 columns per core
P5 = L - 5 + 1                    # 899996 valid positions for K=5
CHUNK = 112500                    # positions per core (8*112500 >= P5)
CSTART_MAX = P5 - S               # 899964 max column start

ACC_COLS = N_GROUPS * 4           # 300: two accum cols per (group, batch)
SUR_COLS = N_GROUPS * 2 * SUR     # 39300 surplus columns per core


def _build_bass(n_groups=N_GROUPS, in_dt=mybir.dt.bfloat16):
    """Build the SPMD Bass module (same program on all cores).

    Per (group, batch): 4 PSUM tiles widths TWS (T0..T3; 2-bank slots, 8
    banks total, 4-slot rotation). ScalarE copies T0->cb0, T2->cb2 (bf16);
    DVE runs two independent tensor_tensor_scan(max, max) ops -- each
    consumes one PSUM and one SBUF element per cycle; each scan broadcast-
    writes its state onto one acc cell (last write = that pair's max).
    """
    nc = bacc.Bacc("TRN2", target_bir_lowering=False, debug=False,
                   num_devices=N_CORES)
    ncol = NCOL
    rhs_d = nc.dram_tensor("rhs", [TROWS, ncol], in_dt, kind="ExternalInput")
    wts_d = nc.dram_tensor("wts", [TROWS, n_groups * 128], in_dt,
                           kind="ExternalInput")
    acc_d = nc.dram_tensor("acc", [128, n_groups * 4], mybir.dt.float32,
                           kind="ExternalOutput")
    sur_d = nc.dram_tensor("sur", [128, n_groups * 2 * SUR],
                           mybir.dt.bfloat16, kind="ExternalOutput")

    bf16 = mybir.dt.bfloat16
    MAX = mybir.AluOpType.max

    with TileContext(nc) as tc:
        with tc.tile_pool(name="io", bufs=1) as io_pool, \
             tc.tile_pool(name="cb", bufs=4) as c_pool, \
             tc.tile_pool(name="ps", bufs=4, space="PSUM") as psum_pool:
            # Inputs split so the first (g,b)'s matmuls only wait on small
            # DMAs: per-batch rhs tiles and a head/tail wts split.
            WSPL = 10 * 128
            rhs0 = io_pool.tile([TROWS, NCOL_B], in_dt)
            rhs1 = io_pool.tile([TROWS, NCOL_B], in_dt)
            rhs_t = [rhs0, rhs1]
            wts_a = io_pool.tile([TROWS, WSPL], in_dt)
            wts_b = io_pool.tile([TROWS, n_groups * 128 - WSPL], in_dt)
            acc = io_pool.tile([128, n_groups * 4], mybir.dt.float32)
            nc.sync.dma_start(wts_a[:, :], wts_d[:, :WSPL])
            nc.sync.dma_start(rhs_t[0][:, :], rhs_d[:, :NCOL_B])
            nc.sync.dma_start(rhs_t[1][:, :], rhs_d[:, NCOL_B:])
            nc.sync.dma_start(wts_b[:, :], wts_d[:, WSPL:])

            gbs = [(g, b) for g in range(n_groups) for b in range(2)]

            def mm_tile(ps, g, b, coff, tw):
                if g * 128 < WSPL:
                    lhsT = wts_a[:, g * 128:(g + 1) * 128]
                else:
                    lhsT = wts_b[:, g * 128 - WSPL:(g + 1) * 128 - WSPL]
                rhs = rhs_t[b]
                for jo, jn in ((0, 512), (512, tw - 512)):
                    nc.tensor.matmul(ps[:, jo:jo + jn], lhsT,
                                     rhs[:, coff + jo:coff + jo + jn],
                                     start=True, stop=True)

            # Software-pipelined: iter k drains gb k-1 (copies first), then
            # interleaves gb k's A-matmuls with gb k-1's S-matmuls so PE
            # visits tiles in dependency-readiness order: A1_k's slot frees
            # when copy1_{k-1} ends (early), A2_k's when copy2_{k-1} ends
            # (period-end) — A2 goes last so it never blocks the S-matmuls
            # the scans need.
            init = -3.0e38

            def emit_a1(g, b):
                a1 = psum_pool.tile([128, TWS[0]], mybir.dt.float32, tag="ps")
                mm_tile(a1, g, b, 0, TWS[0])
                return a1

            def emit_a2(g, b):
                a2 = psum_pool.tile([128, TWS[1]], mybir.dt.float32, tag="ps")
                mm_tile(a2, g, b, TWS[0], TWS[1])
                return a2

            pend = (emit_a1(*gbs[0]), emit_a2(*gbs[0])) + gbs[0]
            for k in range(1, len(gbs) + 1):
                a1, a2, g, b = pend
                c0 = g * 2 + b
                cb = c_pool.tile([128, AW], bf16, tag="cbuf")
                nc.scalar.copy(cb[:, :TWS[0]], a1[:, :])
                nc.scalar.copy(cb[:, TWS[0]:], a2[:, :])
                # Act surplus: ship to DRAM (gpsimd SWDGE queue; SP and the
                # hwdge queues would serialize behind the copy2 sem wait),
                # host takes the max
                nc.gpsimd.dma_start(sur_d[:, c0 * SUR:(c0 + 1) * SUR],
                                    cb[:, SW:AW])
                if k < len(gbs):
                    na1 = emit_a1(*gbs[k])
                s1 = psum_pool.tile([128, TWS[2]], mybir.dt.float32, tag="ps")
                mm_tile(s1, g, b, AW, TWS[2])
                s2 = psum_pool.tile([128, TWS[3]], mybir.dt.float32, tag="ps")
                mm_tile(s2, g, b, AW + TWS[2], TWS[3])
                if k < len(gbs):
                    pend = (na1, emit_a2(*gbs[k])) + gbs[k]
                dst0 = acc[:, 2 * c0:2 * c0 + 1]
                nc.vector.tensor_tensor_scan(
                    dst0.broadcast_to([128, TWS[2]]),
                    s1[:, :], cb[:, :TWS[2]], init, op0=MAX, op1=MAX)
                dst1 = acc[:, 2 * c0 + 1:2 * c0 + 2]
                nc.vector.tensor_tensor_scan(
                    dst1.broadcast_to([128, TWS[3]]),
                    s2[:, :], cb[:, TWS[2]:SW], init, op0=MAX, op1=MAX)

            nc.sync.dma_start(acc_d[:, :], acc[:, :])
    nc.compile()
    return nc


# ---------------- host-side preparation ----------------

def _build_stationary(w1, w2, w3):
    """[TROWS, N_GROUPS*128]: group g covers filters 4g..4g+3 of its conv,
    column m = f_local*32 + r, entry [r+k, m] = w[f, 0, k]."""
    ws = np.zeros((TROWS, N_GROUPS * 128), np.float32)
    convs = [(np.asarray(w1, np.float32), 3),
             (np.asarray(w2, np.float32), 4),
             (np.asarray(w3, np.float32), 5)]
    g = 0
    for w, K in convs:
        for g_local in range(N_FILT // GF):
            for fl in range(GF):
                f = g_local * GF + fl
                for r in range(S):
                    ws[r:r + K, g * 128 + fl * S + r] = w[f, 0, :]
            g += 1
    return ws


def _column_starts(core):
    base = core * CHUNK
    starts = base + S * np.arange(NCOL_B)
    return np.minimum(starts, CSTART_MAX)


def _make_rhs(sig, core, dtype):
    """sig: [B, L] fp32 -> [TROWS, 2*NCOL_B] im2col for this core."""
    starts = _column_starts(core)
    cols = []
    for b in range(B):
        win = np.lib.stride_tricks.sliding_window_view(sig[b], TROWS)
        cols.append(win[starts].T)          # [TROWS, NCOL_B]
    return np.ascontiguousarray(np.concatenate(cols, axis=1)).astype(dtype)


_CACHE = {}


def _get_nc():
    if "nc" not in _CACHE:
        _CACHE["nc"] = _build_bass()
    return _CACHE["nc"]


def _device_acc(rhs_list, wts):
    """Run the bass kernel on 8 cores. rhs_list[i]: [TROWS, 2*NCOL_B].
    Returns list of (acc [128, ACC_COLS] fp32, sur [128, SUR_COLS] bf16)."""
    if os.environ.get("KERNEL_EMULATE"):
        out = []
        for rhs in rhs_list:
            acc = np.empty((128, ACC_COLS), np.float32)
            sur = np.empty((128, SUR_COLS), BF16)
            for g in range(N_GROUPS):
                pg = np.einsum("tm,tn->mn",
                               wts[:, g * 128:(g + 1) * 128].astype(np.float32),
                               rhs.astype(np.float32))  # [128, 2*NCOL_B]
                for b in range(2):
                    c0 = g * 2 + b
                    seg = pg[:, b * NCOL_B:(b + 1) * NCOL_B]
                    cb = seg[:, :AW].astype(BF16).astype(np.float32)
                    acc[:, 2 * c0] = np.maximum(
                        seg[:, AW:AW + TWS[2]].max(axis=1),
                        cb[:, :TWS[2]].max(axis=1))
                    acc[:, 2 * c0 + 1] = np.maximum(
                        seg[:, AW + TWS[2]:].max(axis=1),
                        cb[:, TWS[2]:SW].max(axis=1))
                    sur[:, c0 * SUR:(c0 + 1) * SUR] = cb[:, SW:AW]
            out.append((acc, sur))
        return out

    nc = _get_nc()
    in_maps = [{"rhs": rhs, "wts": wts} for rhs in rhs_list]
    res = bass_utils.run_bass_kernel_spmd(nc, in_maps,
                                          core_ids=list(range(N_CORES)))
    return [(r["acc"], r["sur"]) for r in res.results]


def kernel(x, emb, w1, b1, w2, b2, w3, b3, fc_w, fc_b):
    x = np.asarray(x)
    emb = np.asarray(emb, np.float32)
    sig = emb[x.reshape(-1)].reshape(B, L)          # [2, 900000] fp32

    wts = _build_stationary(w1, w2, w3).astype(BF16)
    rhs_list = [_make_rhs(sig, c, BF16) for c in range(N_CORES)]

    accs = _device_acc(rhs_list, wts)

    # acc cells + DMA'd surplus columns -> per-batch per-filter maxes
    conv_max = np.full((B, 3 * N_FILT), -np.inf, np.float32)
    for acc, sur in accs:
        a = acc.reshape(128, N_GROUPS, 2, 2)
        sm = sur.astype(np.float32).reshape(128, N_GROUPS, 2, SUR).max(axis=3)
        for b in range(B):
            mb = np.maximum(a[:, :, b, :].max(axis=2), sm[:, :, b])  # [128, 75]
            # rows m = f_local*32 + r -> [GF, S, N_GROUPS] -> max over r
            mb = mb.reshape(GF, S, N_GROUPS).max(axis=1)           # [GF, 75]
            # filter id = group_base + (g_local*GF + f_local)
            mb = mb.T.reshape(3, N_FILT // GF, GF).reshape(3 * N_FILT)
            conv_max[b] = np.maximum(conv_max[b], mb)

    # ragged tail positions not covered on device (fp32 host math)
    w1a = np.asarray(w1, np.float32)
    w2a = np.asarray(w2, np.float32)
    for b in range(B):
        for p in (L - 3 + 1 - 1, L - 3 + 1 - 2):   # 899997, 899996 (K=3)
            if p > P5 - 1:
                v = sig[b, p:p + 3] @ w1a[:, 0, :].T
                conv_max[b, :N_FILT] = np.maximum(conv_max[b, :N_FILT], v)
        p = L - 4 + 1 - 1                           # 899996 (K=4)
        if p > P5 - 1:
            v = sig[b, p:p + 4] @ w2a[:, 0, :].T
            conv_max[b, N_FILT:2 * N_FILT] = \
                np.maximum(conv_max[b, N_FILT:2 * N_FILT], v)

    bias = np.concatenate([np.asarray(b1, np.float32),
                           np.asarray(b2, np.float32),
                           np.asarray(b3, np.float32)])
    feats = np.maximum(conv_max + bias[None, :], 0.0)
    out = feats @ np.asarray(fc_w, np.float32).T + np.asarray(fc_b, np.float32)
    return out.astype(np.float32)

